# revision 3
# baseline (speedup 1.0000x reference)
"""Trainium2 Bass kernel for nn_Attention (B=4, S=2048, HIDDEN=768, 12 heads).

Sharding: 8 cores = 4 batches x 2 head-groups (6 heads each). Projection
weights are sliced per head-group and pre-transposed on the host; the
1/sqrt(64) scale is folded into Wq. Each core computes a partial output
(its head-group's contribution through Wo, with bo/2 bias); the host sums
the two partials per batch.

Per-core device program:
  q,k  : [384, S] via matmuls with pre-transposed weights (o on partitions)
  vT   : [S, 390] computed directly transposed; an augmented all-zero weight
         column with bias 1.0 appends a ones-column per head (col 65h+64)
  scores S_c = k_chunk^T q -> PSUM [128, QT]   (k on partitions, q on free)
  E_c = exp(S_c) on ScalarE, PSUM -> SBUF      (no max subtraction; |s| < ~3)
  A  += [vT_c | 1]^T @ E_c -> PSUM [65, QT]    (row 64 = softmax denominator)
  attn = A[0:64] * bcast(1/A[64])              (DVE recip + bcast DMA + mult)
  out_partial = WoT_g^T @ attn + bo/2
"""

import numpy as np

HIDDEN = 768
NUM_HEADS = 12
D = 64
B = 4
S = 2048
P = 128

H_CORE = 6          # heads per core
OC = H_CORE * D     # 384 output channels per core for q/k
WAUG = H_CORE * (D + 1)  # 390: v columns with interleaved ones-columns
C_CHUNKS = HIDDEN // P   # 6
QT = 1024           # q-tile (free dim) for the attention inner loop
MMF = 512           # max fp32 moving free dim per matmul

_CACHE = {}


def _build(with_mask: bool):
    import concourse.bass as bass
    import concourse.tile as tile
    from concourse import bacc, mybir
    from contextlib import ExitStack

    f32 = mybir.dt.float32
    AF = mybir.ActivationFunctionType
    ALU = mybir.AluOpType

    nc = bacc.Bacc(
        "TRN2",
        target_bir_lowering=False,
        debug=False,
        enable_asserts=True,
        num_devices=8,
    )

    x_d = nc.dram_tensor("x", (HIDDEN, S), f32, kind="ExternalInput").ap()
    wq_d = nc.dram_tensor("wqT", (HIDDEN, OC), f32, kind="ExternalInput").ap()
    bq_d = nc.dram_tensor("bq", (OC,), f32, kind="ExternalInput").ap()
    wk_d = nc.dram_tensor("wkT", (HIDDEN, OC), f32, kind="ExternalInput").ap()
    bk_d = nc.dram_tensor("bk", (OC,), f32, kind="ExternalInput").ap()
    wv_d = nc.dram_tensor("wvT", (HIDDEN, WAUG), f32, kind="ExternalInput").ap()
    bvb_d = nc.dram_tensor("bvb", (P, WAUG), f32, kind="ExternalInput").ap()
    wo_d = nc.dram_tensor("woT", (OC, HIDDEN), f32, kind="ExternalInput").ap()
    bo_d = nc.dram_tensor("bo", (HIDDEN,), f32, kind="ExternalInput").ap()
    if with_mask:
        em_d = nc.dram_tensor("expmask", (S, S), f32, kind="ExternalInput").ap()
    out_d = nc.dram_tensor("out", (HIDDEN, S), f32, kind="ExternalOutput").ap()

    QTILES = OC // P      # 3 q/k sbuf tiles
    STILES = S // P       # 16 s-position chunks
    NQH = S // QT         # 2 q-halves
    NF = QT // MMF        # 2 matmul free-slices per QT

    x_r = x_d.rearrange("(t p) s -> p t s", p=P)
    wq_r = wq_d.rearrange("(t p) o -> p t o", p=P)
    wk_r = wk_d.rearrange("(t p) o -> p t o", p=P)
    wv_r = wv_d.rearrange("(t p) o -> p t o", p=P)
    wo_r = wo_d.rearrange("(t p) o -> p t o", p=P)
    bq_r = bq_d.rearrange("(t p) -> p t", p=P)
    bk_r = bk_d.rearrange("(t p) -> p t", p=P)
    bo_r = bo_d.rearrange("(t p) -> p t", p=P)
    out_r = out_d.rearrange("(t p) s -> p t s", p=P)

    with tile.TileContext(nc) as tc, ExitStack() as ctx:
        consts = ctx.enter_context(tc.tile_pool(name="consts", bufs=1))
        persist = ctx.enter_context(tc.tile_pool(name="persist", bufs=1))

        bq_t = consts.tile([P, QTILES], f32)
        nc.sync.dma_start(bq_t[:], bq_r)
        bk_t = consts.tile([P, QTILES], f32)
        nc.sync.dma_start(bk_t[:], bk_r)
        bo_t = consts.tile([P, C_CHUNKS], f32)
        nc.sync.dma_start(bo_t[:], bo_r)
        bvb_t = consts.tile([P, WAUG], f32)
        nc.sync.dma_start(bvb_t[:], bvb_d)
        wo_t = consts.tile([P, QTILES, HIDDEN], f32)
        nc.sync.dma_start(wo_t[:], wo_r)

        q_t = persist.tile([P, QTILES, S], f32)
        k_t = persist.tile([P, QTILES, S], f32)
        v_t = persist.tile([P, STILES, WAUG], f32)
        attn_t = persist.tile([P, QTILES, S], f32)

        # ---------------- phase A: projections ----------------
        with (
            tc.tile_pool(name="phA", bufs=1) as phA,
            tc.tile_pool(name="psA", bufs=2, space="PSUM") as psA,
        ):
            x_t = phA.tile([P, C_CHUNKS, S], f32)
            for c in range(C_CHUNKS):
                nc.sync.dma_start(x_t[:, c, :], x_r[:, c, :])
            wq_t = phA.tile([P, C_CHUNKS, OC], f32)
            nc.sync.dma_start(wq_t[:], wq_r)
            wk_t = phA.tile([P, C_CHUNKS, OC], f32)
            nc.sync.dma_start(wk_t[:], wk_r)
            wv_t = phA.tile([P, C_CHUNKS, WAUG], f32)
            nc.sync.dma_start(wv_t[:], wv_r)

            # q, k projections: out[o_tile(128), s] = sum_c WT[c,o]^T x[c,s]
            for dst, w_sb, b_sb in ((q_t, wq_t, bq_t), (k_t, wk_t, bk_t)):
                for ot in range(QTILES):
                    for half in range(S // QT):
                        ps = psA.tile([P, QT], f32, tag="Pq")
                        for c in range(C_CHUNKS):
                            for nf in range(NF):
                                nc.tensor.matmul(
                                    ps[:, nf * MMF:(nf + 1) * MMF],
                                    wq := w_sb[:, c, ot * P:(ot + 1) * P],
                                    x_t[:, c, half * QT + nf * MMF:
                                        half * QT + (nf + 1) * MMF],
                                    start=(c == 0),
                                    stop=(c == C_CHUNKS - 1),
                                )
                        nc.vector.tensor_scalar_add(
                            dst[:, ot, half * QT:(half + 1) * QT],
                            ps[:],
                            b_sb[:, ot:ot + 1],
                        )

            # vT projection: out[s_tile(128), 390] = sum_c x[c,s]^T WvT[c,:]
            for st in range(STILES):
                ps = psA.tile([P, WAUG], f32, tag="Pv")
                for c in range(C_CHUNKS):
                    nc.tensor.matmul(
                        ps[:],
                        x_t[:, c, st * P:(st + 1) * P],
                        wv_t[:, c, :],
                        start=(c == 0),
                        stop=(c == C_CHUNKS - 1),
                    )
                nc.vector.tensor_tensor(
                    v_t[:, st, :], ps[:], bvb_t[:], ALU.add
                )

        # ---------------- phase B: attention ----------------
        with (
            tc.tile_pool(name="phB", bufs=3) as phB,
            tc.tile_pool(name="psB", bufs=2, space="PSUM") as psB,
            tc.tile_pool(name="outp", bufs=2) as outp,
        ):
            if with_mask:
                em_r = em_d  # [S, S] (k, q)

            for hp in range(H_CORE // 2):
                heads = (2 * hp, 2 * hp + 1)
                for qh in range(NQH):
                    accs = [
                        psB.tile([D + 1, QT], f32, tag="A", name=f"acc{i}")
                        for i in range(2)
                    ]
                    for c in range(STILES):
                        etiles = []
                        for hi, h in enumerate(heads):
                            pb = 64 * (h % 2)
                            sc = psB.tile([P, QT], f32, tag="S")
                            for nf in range(NF):
                                nc.tensor.matmul(
                                    sc[:, nf * MMF:(nf + 1) * MMF],
                                    k_t[pb:pb + D, h // 2, c * P:(c + 1) * P],
                                    q_t[pb:pb + D, h // 2,
                                        qh * QT + nf * MMF:
                                        qh * QT + (nf + 1) * MMF],
                                    start=True,
                                    stop=True,
                                )
                            e = phB.tile([P, QT], f32, tag="E")
                            nc.scalar.activation(e[:], sc[:], AF.Exp)
                            if with_mask:
                                em = phB.tile([P, QT], f32, tag="M")
                                nc.sync.dma_start(
                                    em[:],
                                    em_r[c * P:(c + 1) * P,
                                         qh * QT:(qh + 1) * QT],
                                )
                                nc.vector.tensor_tensor(
                                    e[:], e[:], em[:], ALU.mult
                                )
                            etiles.append(e)
                        for hi, h in enumerate(heads):
                            for nf in range(NF):
                                nc.tensor.matmul(
                                    accs[hi][:, nf * MMF:(nf + 1) * MMF],
                                    v_t[:, c, 65 * h:65 * h + 65],
                                    etiles[hi][:, nf * MMF:(nf + 1) * MMF],
                                    start=(c == 0),
                                    stop=(c == STILES - 1),
                                )
                    for hi, h in enumerate(heads):
                        r = phB.tile([1, QT], f32, tag="r")
                        nc.vector.reciprocal(r[:], accs[hi][D:D + 1, :])
                        bc = phB.tile([D, QT], f32, tag="B")
                        nc.gpsimd.partition_broadcast(bc[:], r[0:1, :])
                        pb = 64 * (h % 2)
                        nc.vector.tensor_tensor(
                            attn_t[pb:pb + D, h // 2, qh * QT:(qh + 1) * QT],
                            accs[hi][0:D, :],
                            bc[:],
                            ALU.mult,
                        )

            # ---------------- output projection ----------------
            for qh in range(NQH):
                for ot in range(C_CHUNKS):
                    ps = psB.tile([P, QT], f32, tag="S")
                    for ct in range(QTILES):
                        for nf in range(NF):
                            nc.tensor.matmul(
                                ps[:, nf * MMF:(nf + 1) * MMF],
                                wo_t[:, ct, ot * P:(ot + 1) * P],
                                attn_t[:, ct, qh * QT + nf * MMF:
                                       qh * QT + (nf + 1) * MMF],
                                start=(ct == 0),
                                stop=(ct == QTILES - 1),
                            )
                    o_sb = outp.tile([P, QT], f32, tag="O")
                    nc.vector.tensor_scalar_add(
                        o_sb[:], ps[:], bo_t[:, ot:ot + 1]
                    )
                    nc.sync.dma_start(
                        out_r[:, ot, qh * QT:(qh + 1) * QT], o_sb[:]
                    )

    nc.compile()
    return nc


def _get_program(with_mask: bool):
    key = ("prog", with_mask)
    if key not in _CACHE:
        _CACHE[key] = _build(with_mask)
    return _CACHE[key]


def _prep_inputs(hidden_state, mask, Wq, bq, Wk, bk, Wv, bv, Wo, bo):
    """Build the 8 per-core input dicts (host-side shard + weight prep)."""
    f = np.float32
    scale = np.float32(D ** -0.5)
    with_mask = bool(np.any(mask))

    in_maps = []
    for b in range(B):
        x_b = np.ascontiguousarray(hidden_state[b, :, 0, :], dtype=f)
        if with_mask:
            em_b = np.exp(mask[b, :, 0, :].astype(f))
        for g in range(2):
            rows = slice(OC * g, OC * (g + 1))
            wqT = np.ascontiguousarray((Wq[rows, :] * scale).T, dtype=f)
            bqs = np.ascontiguousarray(bq[rows] * scale, dtype=f)
            wkT = np.ascontiguousarray(Wk[rows, :].T, dtype=f)
            bks = np.ascontiguousarray(bk[rows], dtype=f)
            # augmented v weights: col 65h+j = Wv row, col 65h+64 = 0 (bias 1)
            wvT = np.zeros((HIDDEN, WAUG), dtype=f)
            bvb = np.zeros((WAUG,), dtype=f)
            for h in range(H_CORE):
                wvT[:, 65 * h:65 * h + 64] = Wv[OC * g + D * h:
                                                OC * g + D * h + D, :].T
                bvb[65 * h:65 * h + 64] = bv[OC * g + D * h:OC * g + D * h + D]
                bvb[65 * h + 64] = 1.0
            woT = np.ascontiguousarray(Wo[:, rows].T, dtype=f)
            m = {
                "x": x_b,
                "wqT": wqT,
                "bq": bqs,
                "wkT": wkT,
                "bk": bks,
                "wvT": wvT,
                "bvb": np.broadcast_to(bvb, (P, WAUG)).copy(),
                "woT": woT,
                "bo": (bo.astype(f) * np.float32(0.5)),
            }
            if with_mask:
                m["expmask"] = em_b
            in_maps.append(m)
    return in_maps, with_mask


def run(inputs: dict, trace: bool = False):
    """Run on 8 NeuronCores; returns (full_output, exec_time_ns_or_None)."""
    from concourse import bass_utils

    in_maps, with_mask = _prep_inputs(**inputs)
    nc = _get_program(with_mask)
    res = bass_utils.run_bass_kernel_spmd(
        nc, in_maps, core_ids=list(range(8)), trace=trace
    )
    out = np.empty((B, HIDDEN, 1, S), dtype=np.float32)
    for b in range(B):
        out[b, :, 0, :] = res.results[2 * b]["out"] + res.results[2 * b + 1]["out"]
    return out, res.exec_time_ns


def kernel(**inputs) -> np.ndarray:
    out, _ = run(inputs, trace=False)
    return out
